# revision 3
# baseline (speedup 1.0000x reference)
"""PhysNet GNN message passing on 8 trn2 NeuronCores (Bass/Tile SPMD).

Strategy: shard 50000 atoms across 8 cores (6250 each). Pairs are grouped by
destination atom into 128-atom windows (idx_i sorted), padded to a uniform
per-window chunk budget so all cores run one SPMD program. Per block:
dense layers in transposed layout (x_T [F, atoms]); xj rows AllGathered into
a shared 50000x128 table; per 128-pair chunk: indirect-DMA gather of xj rows,
g = descr @ Wg, msg = g*xj, scatter-add via one-hot matmul into window PSUM.
ssp(x)=softplus(x)-log2 is approximated as (silu(kx) - k*log2*tanh^2(cx))/k
(max abs err 8.6e-4; no Softplus/Ln table exists on trn2); the 1/k is folded
into the next layer's weights on the host, so device activations carry a k*
scale.
"""
import sys
sys.path.insert(0, "/opt/trn_rl_repo")
import numpy as np
import concourse.bass as bass
import concourse.bacc as bacc
import concourse.mybir as mybir
import concourse.tile as tile
from concourse import bass_utils
from concourse.masks import make_identity

NC = 8
N_ATOMS = 50000
N_PAIRS = 1000000
NA = N_ATOMS // NC          # 6250 atoms per core
F = 128
K = 64
B = 5
NRI, NRF = 3, 2
P = 128
NW = (NA + P - 1) // P      # 49 windows of 128 atoms
LOG2 = float(np.log(2.0))

# fitted ssp approximation params
KA = 1.04378291
CA = 0.43927521
SQ = float(np.sqrt(KA * LOG2))   # Square scale so C = k*log2*tanh^2

_f32 = mybir.dt.float32
_i32 = mybir.dt.int32

AF = mybir.ActivationFunctionType
OP = mybir.AluOpType

COL_T = 512   # dense col tile


def _ssp_scaled(nc, sp, out_sbuf, z, bias_k=None, bias_c=None, scale=1.0):
    """out = k*ssp(scale*z + b) given ACT biases k*(scale b) pre-mult.
    z may be PSUM or SBUF. bias_k/bias_c are [128,1] APs already scaled by
    k and c respectively (or None). Emits Silu+Tanh+Square(ACT) + sub(DVE)."""
    shp = [z.shape[0], z.shape[1]]
    a_t = sp.tile(shp, _f32, tag="ssp_a")
    b_t = sp.tile(shp, _f32, tag="ssp_b")
    c_t = sp.tile(shp, _f32, tag="ssp_c")
    kw_a = dict(scale=KA * scale) if bias_k is None else dict(scale=KA * scale, bias=bias_k)
    kw_b = dict(scale=CA * scale) if bias_c is None else dict(scale=CA * scale, bias=bias_c)
    nc.scalar.activation(a_t[:], z, AF.Silu, **kw_a)
    nc.scalar.activation(b_t[:], z, AF.Tanh, **kw_b)
    nc.scalar.activation(c_t[:], b_t[:], AF.Square, scale=SQ)
    nc.vector.tensor_tensor(out=out_sbuf, in0=a_t[:], in1=c_t[:], op=OP.subtract)


def build(w_ch):
    """Build the SPMD program. w_ch = uniform chunks per window."""
    nc = bacc.Bacc("TRN2", target_bir_lowering=False, debug=False, num_devices=NC)
    TCH = NW * w_ch                      # chunks per block per core
    x0 = nc.dram_tensor("x0", [P, NA], _f32, kind="ExternalInput")
    descr = nc.dram_tensor("descr", [K, TCH * P], _f32, kind="ExternalInput")
    idxs = nc.dram_tensor("idxs", [P, TCH], _i32, kind="ExternalInput")
    offs = nc.dram_tensor("offs", [P, TCH], _f32, kind="ExternalInput")
    iota = nc.dram_tensor("iota", [P, P], _f32, kind="ExternalInput")
    wall = nc.dram_tensor("wall", [B * 13 * P, P], _f32, kind="ExternalInput")
    wg_all = nc.dram_tensor("wg_all", [B * K, P], _f32, kind="ExternalInput")
    biasT = nc.dram_tensor("biasT", [P, B * 32], _f32, kind="ExternalInput")
    xout = nc.dram_tensor("xout", [B, P, NA], _f32, kind="ExternalOutput")
    xj_full = nc.dram_tensor("xj_full", [N_ATOMS, P], _f32,
                             kind="Internal", addr_space="Shared")

    # weight row-block index within wall (per block): Wi,Wj,Wr1x3,Wr2x3,Wout,Wf1x2,Wf2x2
    def wslice(b, j):
        r = (b * 13 + j) * P
        return wall[r:r + P, :]

    # bias column index within biasT (per block, 32 slots):
    # 0:k*bi 1:c*bi 2:k*bj 3:c*bj 4..9: (k,c)*br1 r=0..2  10..12: k*(br2+fold) r
    # 13: bout 14: u 15..18: (k,c)*bf1 r=0..1  19..20: bf2 r  21: unused
    def bcol(b, j):
        return b * 32 + j

    with tile.TileContext(nc) as tc:
        with tc.tile_pool(name="pers", bufs=1) as pp, \
             tc.tile_pool(name="sp", bufs=2) as sp, \
             tc.tile_pool(name="dp", bufs=2) as dp, \
             tc.tile_pool(name="wp", bufs=2) as wp, \
             tc.tile_pool(name="gp", bufs=3) as gpool, \
             tc.tile_pool(name="ps", bufs=2, space="PSUM") as ps, \
             tc.tile_pool(name="dr", bufs=1, space="DRAM") as dr:
            x_t = pp.tile([P, NA], _f32, tag="x")
            xi_t = pp.tile([P, NA], _f32, tag="xi")
            xjt_t = pp.tile([P, NA], _f32, tag="xjt")
            m_t = pp.tile([P, NA], _f32, tag="m")
            xa_t = m_t  # xa dead before m is written; share the slot
            idx_sb = pp.tile([P, TCH], _i32, tag="idx")
            off_sb = pp.tile([P, TCH], _f32, tag="off")
            iota_sb = pp.tile([P, P], _f32, tag="iota")
            bias_sb = pp.tile([P, B * 32], _f32, tag="bias")
            ident = pp.tile([P, P], _f32, tag="ident")
            nc.sync.dma_start(out=x_t[:], in_=x0[:])
            nc.sync.dma_start(out=idx_sb[:], in_=idxs[:])
            nc.sync.dma_start(out=off_sb[:], in_=offs[:])
            nc.sync.dma_start(out=iota_sb[:], in_=iota[:])
            nc.sync.dma_start(out=bias_sb[:], in_=biasT[:])
            make_identity(nc, ident[:])

            ntile = (NA + COL_T - 1) // COL_T
            tiles = [(t * COL_T, min(COL_T, NA - t * COL_T)) for t in range(ntile)]

            def dense(dst, src, widx, bk, bc, b):
                """dst = k*ssp(src @ W + bias) tile-by-tile (transposed layout)."""
                w_sb = wp.tile([P, P], _f32, tag="w")
                nc.sync.dma_start(out=w_sb[:], in_=wslice(b, widx))
                for (c0, cn) in tiles:
                    z = ps.tile([P, COL_T], _f32, tag="z")
                    nc.tensor.matmul(z[:, :cn], lhsT=w_sb[:], rhs=src[:, c0:c0 + cn],
                                     start=True, stop=True)
                    _ssp_scaled(nc, sp, dst[:, c0:c0 + cn], z[:, :cn],
                                bias_k=bias_sb[:, bcol(b, bk):bcol(b, bk) + 1],
                                bias_c=bias_sb[:, bcol(b, bc):bcol(b, bc) + 1])

            for b in range(B):
                # xa = k*ssp(x)
                for (c0, cn) in tiles:
                    _ssp_scaled(nc, sp, xa_t[:, c0:c0 + cn], x_t[:, c0:c0 + cn])
                dense(xi_t, xa_t, 0, 0, 1, b)   # xi (scaled)
                dense(xjt_t, xa_t, 1, 2, 3, b)  # xj (scaled, transposed)
                # transpose xj to rows and write local shard of the table
                xj_loc = dr.tile([NA, P], _f32, tag="xjloc")
                for w in range(NW):
                    wa = min(P, NA - w * P)
                    tp = ps.tile([P, P], _f32, tag="tp")
                    nc.tensor.transpose(out=tp[:wa, :], in_=xjt_t[:, w * P:w * P + wa],
                                        identity=ident[:])
                    rows = sp.tile([P, P], _f32, tag="rows")
                    nc.vector.tensor_copy(out=rows[:wa, :], in_=tp[:wa, :])
                    nc.sync.dma_start(out=xj_loc[w * P:w * P + wa, :], in_=rows[:wa, :])
                nc.gpsimd.collective_compute(
                    "AllGather", OP.bypass,
                    replica_groups=[list(range(NC))],
                    ins=[xj_loc[:]], outs=[xj_full[:]])

                # pair stream
                wg_sb = wp.tile([K, P], _f32, tag="wg")
                nc.sync.dma_start(out=wg_sb[:], in_=wg_all[b * K:(b + 1) * K, :])
                for w in range(NW):
                    wa = min(P, NA - w * P)
                    dt = dp.tile([K, w_ch * P], _f32, tag="descr")
                    nc.sync.dma_start(
                        out=dt[:], in_=descr[:, w * w_ch * P:(w + 1) * w_ch * P])
                    macc = ps.tile([P, P], _f32, tag="macc")
                    for c in range(w_ch):
                        ci = w * w_ch + c
                        gt = gpool.tile([P, P], _f32, tag="gt")
                        nc.gpsimd.indirect_dma_start(
                            out=gt[:], out_offset=None, in_=xj_full[:],
                            in_offset=bass.IndirectOffsetOnAxis(
                                ap=idx_sb[:, ci:ci + 1], axis=0))
                        gps = ps.tile([P, P], _f32, tag="gps")
                        nc.tensor.matmul(gps[:], lhsT=dt[:, c * P:(c + 1) * P],
                                         rhs=wg_sb[:], start=True, stop=True)
                        s_t = sp.tile([P, P], _f32, tag="s")
                        nc.vector.tensor_scalar(
                            out=s_t[:], in0=iota_sb[:],
                            scalar1=off_sb[:, ci:ci + 1], scalar2=None,
                            op0=OP.is_equal)
                        msg = sp.tile([P, P], _f32, tag="msg")
                        nc.vector.tensor_tensor(out=msg[:], in0=gps[:], in1=gt[:],
                                                op=OP.mult)
                        nc.tensor.matmul(macc[:], lhsT=msg[:], rhs=s_t[:],
                                         start=(c == 0), stop=(c == w_ch - 1))
                    nc.vector.tensor_tensor(
                        out=m_t[:, w * P:w * P + wa], in0=macc[:, :wa],
                        in1=xi_t[:, w * P:w * P + wa], op=OP.add)

                # residual (interaction) x3 — all on m (scaled)
                for r in range(NRI):
                    w1 = wp.tile([P, P], _f32, tag="w")
                    nc.sync.dma_start(out=w1[:], in_=wslice(b, 2 + r))
                    w2 = wp.tile([P, P], _f32, tag="w")
                    nc.sync.dma_start(out=w2[:], in_=wslice(b, 5 + r))
                    for (c0, cn) in tiles:
                        t1 = sp.tile([P, COL_T], _f32, tag="t1")
                        _ssp_scaled(nc, sp, t1[:, :cn], m_t[:, c0:c0 + cn],
                                    scale=1.0 / KA)  # m is k-scaled
                        z1 = ps.tile([P, COL_T], _f32, tag="z")
                        nc.tensor.matmul(z1[:, :cn], lhsT=w1[:], rhs=t1[:, :cn],
                                         start=True, stop=True)
                        t2 = sp.tile([P, COL_T], _f32, tag="t2")
                        _ssp_scaled(nc, sp, t2[:, :cn], z1[:, :cn],
                                    bias_k=bias_sb[:, bcol(b, 4 + 2 * r):bcol(b, 4 + 2 * r) + 1],
                                    bias_c=bias_sb[:, bcol(b, 5 + 2 * r):bcol(b, 5 + 2 * r) + 1])
                        z2 = ps.tile([P, COL_T], _f32, tag="z")
                        nc.tensor.matmul(z2[:, :cn], lhsT=w2[:], rhs=t2[:, :cn],
                                         start=True, stop=True)
                        t3 = sp.tile([P, COL_T], _f32, tag="t3")
                        nc.vector.tensor_scalar(
                            out=t3[:, :cn], in0=z2[:, :cn], scalar1=KA,
                            scalar2=bias_sb[:, bcol(b, 10 + r):bcol(b, 10 + r) + 1],
                            op0=OP.mult, op1=OP.add)
                        nc.vector.tensor_tensor(out=m_t[:, c0:c0 + cn],
                                                in0=m_t[:, c0:c0 + cn],
                                                in1=t3[:, :cn], op=OP.add)
                # x = u*x + ssp(m)@Wout + bout   (ssp(m) scaled; Wout pre-divided)
                wo = wp.tile([P, P], _f32, tag="w")
                nc.sync.dma_start(out=wo[:], in_=wslice(b, 8))
                for (c0, cn) in tiles:
                    mp = sp.tile([P, COL_T], _f32, tag="t1")
                    _ssp_scaled(nc, sp, mp[:, :cn], m_t[:, c0:c0 + cn], scale=1.0 / KA)
                    z = ps.tile([P, COL_T], _f32, tag="z")
                    nc.tensor.matmul(z[:, :cn], lhsT=wo[:], rhs=mp[:, :cn],
                                     start=True, stop=True)
                    ux = sp.tile([P, COL_T], _f32, tag="t2")
                    nc.vector.tensor_scalar(
                        out=ux[:, :cn], in0=x_t[:, c0:c0 + cn],
                        scalar1=bias_sb[:, bcol(b, 14):bcol(b, 14) + 1],
                        scalar2=bias_sb[:, bcol(b, 13):bcol(b, 13) + 1],
                        op0=OP.mult, op1=OP.add)
                    nc.vector.tensor_tensor(out=x_t[:, c0:c0 + cn], in0=ux[:, :cn],
                                            in1=z[:, :cn], op=OP.add)
                # residual (feature) x2 — on x (true scale)
                for r in range(NRF):
                    w1 = wp.tile([P, P], _f32, tag="w")
                    nc.sync.dma_start(out=w1[:], in_=wslice(b, 9 + r))
                    w2 = wp.tile([P, P], _f32, tag="w")
                    nc.sync.dma_start(out=w2[:], in_=wslice(b, 11 + r))  # careful map
                    for (c0, cn) in tiles:
                        t1 = sp.tile([P, COL_T], _f32, tag="t1")
                        _ssp_scaled(nc, sp, t1[:, :cn], x_t[:, c0:c0 + cn])
                        z1 = ps.tile([P, COL_T], _f32, tag="z")
                        nc.tensor.matmul(z1[:, :cn], lhsT=w1[:], rhs=t1[:, :cn],
                                         start=True, stop=True)
                        t2 = sp.tile([P, COL_T], _f32, tag="t2")
                        _ssp_scaled(nc, sp, t2[:, :cn], z1[:, :cn],
                                    bias_k=bias_sb[:, bcol(b, 15 + 2 * r):bcol(b, 15 + 2 * r) + 1],
                                    bias_c=bias_sb[:, bcol(b, 16 + 2 * r):bcol(b, 16 + 2 * r) + 1])
                        z2 = ps.tile([P, COL_T], _f32, tag="z")
                        nc.tensor.matmul(z2[:, :cn], lhsT=w2[:], rhs=t2[:, :cn],
                                         start=True, stop=True)
                        t3 = sp.tile([P, COL_T], _f32, tag="t3")
                        nc.vector.tensor_scalar(
                            out=t3[:, :cn], in0=z2[:, :cn], scalar1=1.0,
                            scalar2=bias_sb[:, bcol(b, 19 + r):bcol(b, 19 + r) + 1],
                            op0=OP.mult, op1=OP.add)
                        nc.vector.tensor_tensor(out=x_t[:, c0:c0 + cn],
                                                in0=x_t[:, c0:c0 + cn],
                                                in1=t3[:, :cn], op=OP.add)
                nc.sync.dma_start(out=xout[b, :, :], in_=x_t[:])
    nc.compile()
    return nc


def kernel(**inputs):
    feats = np.asarray(inputs["features"], np.float32)
    cutoffs = np.asarray(inputs["cutoffs"], np.float32)
    rbfs = np.asarray(inputs["rbfs"], np.float32)
    idx_i = np.asarray(inputs["idx_i"]).astype(np.int64)
    idx_j = np.asarray(inputs["idx_j"]).astype(np.int64)
    W = {k: np.asarray(inputs[k], np.float32) for k in
         ["Wg", "Wi", "bi", "Wj", "bj", "Wr1", "br1", "Wr2", "br2",
          "Wout", "bout", "u", "Wf1", "bf1", "Wf2", "bf2"]}
    CC = -8.9582e-4  # ssp approx constant; folded into consumer biases below

    descr_full = cutoffs[:, None] * rbfs                      # [Pairs, K]

    # ---- shard pairs by destination atom core & window; compute budget ----
    bounds = np.searchsorted(idx_i, np.arange(0, N_ATOMS + 1, NA))
    win_of = (idx_i % NA) // P                               # window within core
    cnts = np.zeros((NC, NW), np.int64)
    for c in range(NC):
        s, e = bounds[c], bounds[c + 1]
        cnts[c] = np.bincount(win_of[s:e], minlength=NW)
    w_ch = int(np.ceil(cnts.max() / P))
    TCH = NW * w_ch

    in_maps = []
    for c in range(NC):
        s, e = bounds[c], bounds[c + 1]
        d = np.zeros((TCH * P, K), np.float32)
        ji = np.zeros((TCH * P,), np.int32)
        of = np.zeros((TCH * P,), np.float32)
        pos = s
        for w in range(NW):
            n = cnts[c, w]
            base = w * w_ch * P
            d[base:base + n] = descr_full[pos:pos + n]
            ji[base:base + n] = idx_j[pos:pos + n]
            of[base:base + n] = (idx_i[pos:pos + n] % NA) - w * P
            pos += n
        # device layouts
        descr_t = np.ascontiguousarray(d.T)                   # [K, TCH*P]
        idx_t = np.ascontiguousarray(ji.reshape(TCH, P).T)    # [P, TCH]
        off_t = np.ascontiguousarray(of.reshape(TCH, P).T)    # [P, TCH]
        x0 = np.ascontiguousarray(feats[c * NA:(c + 1) * NA].T)
        in_maps.append(dict(x0=x0, descr=descr_t, idxs=idx_t, offs=off_t))

    iota = np.broadcast_to(np.arange(P, dtype=np.float32), (P, P)).copy()
    # ---- weights: fold 1/KA into consumers of scaled activations ----
    inv = 1.0 / KA
    wall = np.zeros((B, 13, P, P), np.float32)
    wg_all = np.zeros((B, K, P), np.float32)
    biasT = np.zeros((B, 32, P), np.float32)
    for b in range(B):
        wall[b, 0] = W["Wi"][b] * inv
        wall[b, 1] = W["Wj"][b] * inv
        for r in range(NRI):
            wall[b, 2 + r] = W["Wr1"][b, r] * inv
            wall[b, 5 + r] = W["Wr2"][b, r] * inv
        wall[b, 8] = W["Wout"][b] * inv
        for r in range(NRF):
            wall[b, 9 + r] = W["Wf1"][b, r] * inv
            wall[b, 11 + r] = W["Wf2"][b, r] * inv
        wg_all[b] = W["Wg"][b]
        bi_e = W["bi"][b] + CC * W["Wi"][b].sum(0)
        bj_e = W["bj"][b] + CC * W["Wj"][b].sum(0)
        biasT[b, 0] = KA * bi_e
        biasT[b, 1] = CA * bi_e
        biasT[b, 2] = KA * bj_e
        biasT[b, 3] = CA * bj_e
        for r in range(NRI):
            br1_e = W["br1"][b, r] + CC * W["Wr1"][b, r].sum(0)
            br2_e = W["br2"][b, r] + CC * W["Wr2"][b, r].sum(0)
            biasT[b, 4 + 2 * r] = KA * br1_e
            biasT[b, 5 + 2 * r] = CA * br1_e
            biasT[b, 10 + r] = KA * br2_e
        biasT[b, 13] = W["bout"][b] + CC * W["Wout"][b].sum(0)
        biasT[b, 14] = W["u"][b]
        for r in range(NRF):
            bf1_e = W["bf1"][b, r] + CC * W["Wf1"][b, r].sum(0)
            biasT[b, 15 + 2 * r] = KA * bf1_e
            biasT[b, 16 + 2 * r] = CA * bf1_e
            biasT[b, 19 + r] = W["bf2"][b, r] + CC * W["Wf2"][b, r].sum(0)
    shared = dict(iota=iota,
                  wall=wall.reshape(B * 13 * P, P),
                  wg_all=wg_all.reshape(B * K, P),
                  biasT=np.ascontiguousarray(
                      biasT.reshape(B * 32, P).T))
    for m in in_maps:
        m.update(shared)

    nc = build(w_ch)
    res = bass_utils.run_bass_kernel_spmd(nc, in_maps, core_ids=list(range(NC)))
    global LAST_EXEC_NS, LAST_TRACE, LAST_PROFILE_JSON
    if getattr(res, "exec_time_ns", None):
        LAST_EXEC_NS = res.exec_time_ns
    if getattr(res, "instructions_and_trace", None):
        LAST_TRACE = res.instructions_and_trace[1]
    if getattr(res, "profile_json", None):
        LAST_PROFILE_JSON = res.profile_json
    out = np.empty((B, N_ATOMS, F), np.float32)
    for c in range(NC):
        slab = res.results[c]["xout"]          # [B, 128, NA]
        out[:, c * NA:(c + 1) * NA, :] = np.transpose(slab, (0, 2, 1))
    return out



# revision 34
# speedup vs baseline: 2.0341x; 2.0341x over previous
"""PhysNet GNN message passing on 8 trn2 NeuronCores (Bass/Tile SPMD), v2.

Atoms sharded 8 ways (6250/core; 49 windows of 128). Pairs grouped by
destination window, split lo/hi at atom 32768 (dma_gather int16 reach),
padded to uniform K_LO+K_HI chunks of 128 pairs (pad gathers row 0, with
all-zero one-hot rows). Dense stack in float32r (full-rate PE, ~1e-4 err),
exact shifted-softplus via ACT Exp then Ln(0.5*e+0.5). Pair stream bf16:
per block xj rows AllGathered (bf16) into a 50000x128 table; per window one
dma_gather per half; g = descr_chunk @ Wg (bf16); msg = g * xj (DVE,
batched); scatter-add into window PSUM via matmul against host-precomputed
fp8 one-hot matrices.
"""
import os
import sys
sys.path.insert(0, "/opt/trn_rl_repo")
DBG = set((os.environ.get("KDBG") or "").split(","))
import numpy as np
import ml_dtypes
import concourse.bass as bass
import concourse.bacc as bacc
import concourse.mybir as mybir
import concourse.tile as tile
from concourse import bass_utils
from concourse.library_config import mlp
from concourse.masks import make_identity

NC = 8
N_ATOMS = 50000
N_PAIRS = 1000000
NA = N_ATOMS // NC
F = 128
K = 64
B = 5
NRI, NRF = 3, 2
P = 128
NW = (NA + P - 1) // P
SPLIT = 32768
GM = 4                      # chunks per DVE mult group (gps psum = 1 bank)
GCAP = int(os.environ.get("KGCAP") or 4)   # max chunks per dma_gather call

_f32 = mybir.dt.float32
_f32r = mybir.dt.float32r
_bf16 = mybir.dt.bfloat16
_fp8 = mybir.dt.float8e4
_i16 = mybir.dt.int16

AF = mybir.ActivationFunctionType
OP = mybir.AluOpType

COL_T = 512
ACT_T = 1024

LAST_EXEC_NS = None
LAST_TRACE = None
LAST_PROFILE_JSON = None


def build(k_lo, k_hi):
    w_ch = k_lo + k_hi
    TCH = NW * w_ch
    nc = bacc.Bacc("TRN2", target_bir_lowering=False, debug=False,
                   num_devices=NC, num_swdge_queues=4)
    x0 = nc.dram_tensor("x0", [P, NA], _f32, kind="ExternalInput")
    descr = nc.dram_tensor("descr", [K, TCH * P], _bf16, kind="ExternalInput")
    onehot = nc.dram_tensor("onehot", [TCH * P, P], _fp8, kind="ExternalInput")
    idx16 = nc.dram_tensor("idx16", [P, TCH * 8], _i16, kind="ExternalInput")
    wall = nc.dram_tensor("wall", [B * 13 * P, P], _f32r, kind="ExternalInput")
    wg_all = nc.dram_tensor("wg_all", [B * K, P], _bf16, kind="ExternalInput")
    biasT = nc.dram_tensor("biasT", [P, B * 16], _f32, kind="ExternalInput")
    brow = nc.dram_tensor("brow", [1, B * 2 * P], _f32r, kind="ExternalInput")
    xout = nc.dram_tensor("xout", [B, P, NA], _f32, kind="ExternalOutput")
    xj_loc = nc.dram_tensor("xj_loc", [NA, P], _bf16, kind="Internal")
    xj_full = nc.dram_tensor("xj_full", [N_ATOMS, P], _bf16,
                             kind="Internal", addr_space="Shared")
    # dma_gather (custom Q7 kernel) sources from local DRAM; mirror the
    # AllGather result out of the Shared segment before gathering.
    xj_mir2 = [nc.dram_tensor(f"xj_mir{i}", [N_ATOMS, P], _bf16,
                              kind="Internal") for i in range(2)]

    def wslice(b, j):
        r = (b * 13 + j) * P
        return wall[r:r + P, :]

    # bias cols (16/block): 0:bi 1..3:br1[r] 4..6:br2[r] 7:bout 8:u
    # 9..10:bf1[r] 11..12:bf2[r] 13:const 0.5
    ngrp = (NA + ACT_T - 1) // ACT_T
    grps = [(g * ACT_T, min(ACT_T, NA - g * ACT_T)) for g in range(ngrp)]

    with tile.TileContext(nc) as tc:
        with tc.tile_pool(name="pers", bufs=1) as pp, \
             tc.tile_pool(name="st", bufs=2) as st, \
             tc.tile_pool(name="sp", bufs=2) as sp, \
             tc.tile_pool(name="dp", bufs=2) as dp, \
             tc.tile_pool(name="wp", bufs=2) as wp, \
             tc.tile_pool(name="gp", bufs=2) as gp, \
             tc.tile_pool(name="dbgp", bufs=1) as dbgp, \
             tc.tile_pool(name="zz", bufs=2, space="PSUM") as zz, \
             tc.tile_pool(name="pg", bufs=2, space="PSUM") as pg, \
             tc.tile_pool(name="pm", bufs=2, space="PSUM") as pm:
            nc.gpsimd.load_library(mlp)
            x_t = pp.tile([P, NA], _f32, tag="x")
            xa_t = pp.tile([P, NA], _f32r, tag="xa")
            xi_t = pp.tile([P, NA], _f32, tag="xi")
            m_t = pp.tile([P, NA], _f32, tag="m")
            xjt_t = pp.tile([P, NA], _bf16, tag="xjt")
            ident = pp.tile([P, P], _bf16, tag="ident")
            idx_sb = pp.tile([P, TCH * 8], _i16, tag="idx")
            bias_sb = pp.tile([P, B * 16], _f32, tag="bias")
            brow_sb = pp.tile([1, B * 2 * P], _f32r, tag="brow")
            nc.sync.dma_start(out=x_t[:], in_=x0[:])
            nc.sync.dma_start(out=idx_sb[:], in_=idx16[:])
            nc.sync.dma_start(out=bias_sb[:], in_=biasT[:])
            nc.sync.dma_start(out=brow_sb[:], in_=brow[:])
            make_identity(nc, ident[:])

            def bc(b, j):
                c = b * 16 + j
                return bias_sb[:, c:c + 1]

            half = bias_sb[:, 13:14]  # 0.5 column (block 0)

            def ssp_group(dst_ap, src_ap, gn, bias=None, safe=False):
                """dst[:, :gn] = ssp(src[:, :gn] + bias); src SBUF or PSUM.

                safe=True uses ssp(v) = max(v,0) + Ln(0.5 + 0.5 e^-|v|),
                immune to exp overflow / Ln table range (activations grow
                to ~300 by block 4)."""
                if not safe:
                    e_t = st.tile([P, ACT_T], _f32, tag="sspe")
                    kw = dict(bias=bias) if bias is not None else {}
                    nc.scalar.activation(e_t[:, :gn], src_ap, AF.Exp, **kw)
                    nc.scalar.activation(dst_ap, e_t[:, :gn], AF.Ln,
                                         scale=0.5, bias=half)
                    return
                if bias is not None:
                    zb = st.tile([P, ACT_T], _f32, tag="zb")
                    nc.vector.tensor_scalar(out=zb[:, :gn], in0=src_ap,
                                            scalar1=bias, scalar2=None,
                                            op0=OP.add)
                    zb_ap = zb[:, :gn]
                else:
                    zb_ap = src_ap
                a_t = st.tile([P, ACT_T], _f32, tag="sspe")
                nc.scalar.activation(a_t[:, :gn], zb_ap, AF.Abs)
                e_t = st.tile([P, ACT_T], _f32, tag="sspe")
                nc.scalar.activation(e_t[:, :gn], a_t[:, :gn], AF.Exp,
                                     scale=-1.0)
                l_t = st.tile([P, ACT_T], _f32, tag="sspe")
                nc.scalar.activation(l_t[:, :gn], e_t[:, :gn], AF.Ln,
                                     scale=0.5, bias=half)
                nc.vector.scalar_tensor_tensor(
                    out=dst_ap, in0=zb_ap, scalar=0.0, in1=l_t[:, :gn],
                    op0=OP.max, op1=OP.add)

            def mm_group(z, w_sb, src, src0, gn):
                nct = (gn + COL_T - 1) // COL_T
                for t in range(nct):
                    o = t * COL_T
                    cn = min(COL_T, gn - o)
                    nc.tensor.matmul(z[:, o:o + cn], lhsT=w_sb[:],
                                     rhs=src[:, src0 + o:src0 + o + cn],
                                     start=True, stop=True)

            for b in range(1 if "dump" in DBG else B):
                sf = b >= 2
                # xa = ssp(x)  (f32r)
                for (g0, gn) in grps:
                    ssp_group(xa_t[:, g0:g0 + gn], x_t[:, g0:g0 + gn], gn,
                              safe=sf)
                if "dump" in DBG:
                    nc.sync.dma_start(out=xout[0, :, :],
                                      in_=xa_t[:].bitcast(_f32))
                # xj in transposed layout (like xi), then PE-transpose
                wj_sb = wp.tile([P, P], _f32r, tag="w")
                nc.sync.dma_start(out=wj_sb[:], in_=wslice(b, 1))
                for (g0, gn) in grps:
                    z = zz.tile([P, ACT_T], _f32, tag="z")
                    mm_group(z, wj_sb, xa_t, g0, gn)
                    ssp_group(xjt_t[:, g0:g0 + gn], z[:, :gn], gn,
                              bias=bc(b, 14), safe=sf)
                for w in range(NW):
                    wa = min(P, NA - w * P)
                    tp_ps = pm.tile([P, P], _f32, tag="macc")
                    tp_bf = tp_ps[:].bitcast(_bf16)[:, 0:P]
                    nc.tensor.transpose(out=tp_bf[:wa, :],
                                        in_=xjt_t[:, w * P:w * P + wa],
                                        identity=ident[:])
                    rows = st.tile([P, P], _bf16, tag="sspw")
                    nc.vector.tensor_copy(out=rows[:wa, :], in_=tp_bf[:wa, :])
                    nc.sync.dma_start(out=xj_loc[w * P:w * P + wa, :],
                                      in_=rows[:wa, :])
                if "nocc" in DBG:
                    nc.sync.dma_start(out=xj_mir2[b % 2][0:NA, :], in_=xj_loc[:])
                else:
                    nc.gpsimd.collective_compute(
                        "AllGather", OP.bypass,
                        replica_groups=[list(range(NC))],
                        ins=[xj_loc[:]], outs=[xj_full[:]])
                    nc.sync.dma_start(out=xj_mir2[b % 2][:], in_=xj_full[:])
                # xi = ssp(xa @ Wi + bi)
                wi_sb = wp.tile([P, P], _f32r, tag="w")
                nc.sync.dma_start(out=wi_sb[:], in_=wslice(b, 0))
                for (g0, gn) in grps:
                    z = zz.tile([P, ACT_T], _f32, tag="z")
                    mm_group(z, wi_sb, xa_t, g0, gn)
                    ssp_group(xi_t[:, g0:g0 + gn], z[:, :gn], gn,
                              bias=bc(b, 0), safe=sf)
                if "dump" in DBG:
                    nc.sync.dma_start(out=xout[1, :, :], in_=xi_t[:])


                # ---- pair stream (1-stage software pipeline per window) ----
                wg_sb = wp.tile([K, P], _bf16, tag="wg")
                nc.sync.dma_start(out=wg_sb[:], in_=wg_all[b * K:(b + 1) * K, :])
                ngm = (w_ch + GM - 1) // GM
                for w in range(NW):
                    wa = min(P, NA - w * P)
                    gt = gp.tile([P, w_ch, P], _bf16, tag="gt")
                    cbase = w * w_ch
                    if "nogather" in DBG:
                        nc.vector.memset(gt[:].rearrange("p c f -> p (c f)"), 1.0)
                    else:
                        # sub-split calls to <=GCAP chunks (SWDGE ring budget)
                        qn = 0
                        xj_mir = xj_mir2[b % 2]
                        for h0, hk, src in ((0, k_lo, xj_mir[0:SPLIT, :]),
                                            (k_lo, k_hi,
                                             xj_mir[SPLIT:N_ATOMS, :])):
                            for s0 in range(0, hk, GCAP):
                                sk = min(GCAP, hk - s0)
                                c0 = cbase + h0 + s0
                                nc.gpsimd.dma_gather(
                                    gt[:, h0 + s0:h0 + s0 + sk, :], src,
                                    idx_sb[:, c0 * 8:(c0 + sk) * 8],
                                    sk * P, sk * P, P,
                                    queue_num=0 if "oneq" in DBG
                                    else (4 * w + qn) % 4)
                                qn += 1
                    dt = dp.tile([K, w_ch * P], _bf16, tag="descr")
                    nc.sync.dma_start(
                        out=dt[:], in_=descr[:, cbase * P:(cbase + w_ch) * P])
                    oh = dp.tile([P, w_ch, P], _fp8, tag="oh")
                    nc.sync.dma_start(
                        out=oh[:],
                        in_=onehot[cbase * P:(cbase + w_ch) * P, :]
                        .rearrange("(c p) s -> p c s", p=P))
                    macc = pm.tile([P, P], _f32, tag="macc")
                    msgs = []
                    for g in range(ngm):
                        c0 = g * GM
                        cn = min(GM, w_ch - c0)
                        gps = pg.tile([P, GM * P], _f32, tag="gps")
                        for c in range(c0, c0 + cn):
                            nc.tensor.matmul(
                                gps[:, (c - c0) * P:(c - c0 + 1) * P],
                                lhsT=dt[:, c * P:(c + 1) * P], rhs=wg_sb[:],
                                start=True, stop=True)
                        msg = sp.tile([P, GM * P], _bf16, tag="msg")
                        nc.vector.tensor_tensor(
                            out=msg[:, :cn * P], in0=gps[:, :cn * P],
                            in1=gt[:, c0:c0 + cn, :].rearrange(
                                "p c f -> p (c f)"),
                            op=OP.mult)
                        msgs.append((c0, cn, msg))
                        # drain scatter for the PREVIOUS group (pipeline)
                        if len(msgs) == 2:
                            pc0, pcn, pmsg = msgs.pop(0)
                            for c in range(pc0, pc0 + pcn):
                                nc.tensor.matmul(
                                    macc[:],
                                    lhsT=pmsg[:, (c - pc0) * P:(c - pc0 + 1) * P],
                                    rhs=oh[:, c, :],
                                    start=(c == 0), stop=False)
                    pc0, pcn, pmsg = msgs.pop(0)
                    for c in range(pc0, pc0 + pcn):
                        nc.tensor.matmul(
                            macc[:],
                            lhsT=pmsg[:, (c - pc0) * P:(c - pc0 + 1) * P],
                            rhs=oh[:, c, :],
                            start=(c == 0), stop=(c == pc0 + pcn - 1))
                    nc.vector.tensor_tensor(
                        out=m_t[:, w * P:w * P + wa], in0=macc[:, :wa],
                        in1=xi_t[:, w * P:w * P + wa], op=OP.add)
                if "dump" in DBG:
                    nc.sync.dma_start(out=xout[2, :, :], in_=m_t[:])

                # ---- residual (interaction) x3 on m ----
                for r in range(NRI):
                    w1 = wp.tile([P, P], _f32r, tag="w")
                    nc.sync.dma_start(out=w1[:], in_=wslice(b, 2 + r))
                    w2 = wp.tile([P, P], _f32r, tag="w")
                    nc.sync.dma_start(out=w2[:], in_=wslice(b, 5 + r))
                    for (g0, gn) in grps:
                        t1 = st.tile([P, ACT_T], _f32r, tag="t1")
                        ssp_group(t1[:, :gn], m_t[:, g0:g0 + gn], gn, safe=sf)
                        z1 = zz.tile([P, ACT_T], _f32, tag="z")
                        mm_group(z1, w1, t1, 0, gn)
                        t2 = st.tile([P, ACT_T], _f32r, tag="t2")
                        ssp_group(t2[:, :gn], z1[:, :gn], gn, bias=bc(b, 1 + r),
                                  safe=sf)
                        z2 = zz.tile([P, ACT_T], _f32, tag="z")
                        mm_group(z2, w2, t2, 0, gn)
                        nc.vector.scalar_tensor_tensor(
                            out=m_t[:, g0:g0 + gn], in0=z2[:, :gn],
                            scalar=bc(b, 4 + r), in1=m_t[:, g0:g0 + gn],
                            op0=OP.add, op1=OP.add)

                # x = u*x + ssp(m) @ Wout + bout
                wo = wp.tile([P, P], _f32r, tag="w")
                nc.sync.dma_start(out=wo[:], in_=wslice(b, 8))
                for (g0, gn) in grps:
                    mp = st.tile([P, ACT_T], _f32r, tag="t1")
                    ssp_group(mp[:, :gn], m_t[:, g0:g0 + gn], gn, safe=sf)
                    z = zz.tile([P, ACT_T], _f32, tag="z")
                    mm_group(z, wo, mp, 0, gn)
                    ux = st.tile([P, ACT_T], _f32, tag="t2")
                    nc.vector.tensor_scalar(
                        out=ux[:, :gn], in0=x_t[:, g0:g0 + gn],
                        scalar1=bc(b, 8), scalar2=None, op0=OP.mult)
                    nc.vector.scalar_tensor_tensor(
                        out=x_t[:, g0:g0 + gn], in0=z[:, :gn],
                        scalar=bc(b, 7), in1=ux[:, :gn],
                        op0=OP.add, op1=OP.add)
                if "dump" in DBG:
                    nc.sync.dma_start(out=xout[3, :, :], in_=x_t[:])

                # ---- residual (feature) x2 on x ----
                for r in range(NRF):
                    w1 = wp.tile([P, P], _f32r, tag="w")
                    nc.sync.dma_start(out=w1[:], in_=wslice(b, 9 + r))
                    w2 = wp.tile([P, P], _f32r, tag="w")
                    nc.sync.dma_start(out=w2[:], in_=wslice(b, 11 + r))
                    for (g0, gn) in grps:
                        t1 = st.tile([P, ACT_T], _f32r, tag="t1")
                        ssp_group(t1[:, :gn], x_t[:, g0:g0 + gn], gn, safe=sf)
                        z1 = zz.tile([P, ACT_T], _f32, tag="z")
                        mm_group(z1, w1, t1, 0, gn)
                        t2 = st.tile([P, ACT_T], _f32r, tag="t2")
                        ssp_group(t2[:, :gn], z1[:, :gn], gn,
                                  bias=bc(b, 9 + r), safe=sf)
                        z2 = zz.tile([P, ACT_T], _f32, tag="z")
                        mm_group(z2, w2, t2, 0, gn)
                        nc.vector.scalar_tensor_tensor(
                            out=x_t[:, g0:g0 + gn], in0=z2[:, :gn],
                            scalar=bc(b, 11 + r), in1=x_t[:, g0:g0 + gn],
                            op0=OP.add, op1=OP.add)
                nc.sync.dma_start(out=xout[b, :, :], in_=x_t[:])
    nc.compile()
    return nc


def kernel(**inputs):
    feats = np.asarray(inputs["features"], np.float32)
    cutoffs = np.asarray(inputs["cutoffs"], np.float32)
    rbfs = np.asarray(inputs["rbfs"], np.float32)
    idx_i = np.asarray(inputs["idx_i"]).astype(np.int64)
    idx_j = np.asarray(inputs["idx_j"]).astype(np.int64)
    W = {k: np.asarray(inputs[k], np.float32) for k in
         ["Wg", "Wi", "bi", "Wj", "bj", "Wr1", "br1", "Wr2", "br2",
          "Wout", "bout", "u", "Wf1", "bf1", "Wf2", "bf2"]}

    descr_full = (cutoffs[:, None] * rbfs).astype(np.float32)

    bounds = np.searchsorted(idx_i, np.arange(0, N_ATOMS + 1, NA))
    per_core = []
    n_lo = np.zeros((NC, NW), np.int64)
    n_hi = np.zeros((NC, NW), np.int64)
    for c in range(NC):
        s, e = bounds[c], bounds[c + 1]
        loc = idx_i[s:e] - c * NA
        win = loc // P
        is_hi = idx_j[s:e] >= SPLIT
        per_core.append((s, e, loc, win, is_hi))
        for w in range(NW):
            msk = win == w
            n_hi[c, w] = np.count_nonzero(msk & is_hi)
            n_lo[c, w] = np.count_nonzero(msk) - n_hi[c, w]
    k_lo = int(np.ceil(n_lo.max() / P))
    k_hi = int(np.ceil(n_hi.max() / P))
    assert k_lo <= 16 and k_hi <= 16, (k_lo, k_hi)
    w_ch = k_lo + k_hi
    TCH = NW * w_ch

    in_maps = []
    for c in range(NC):
        s, e, loc, win, is_hi = per_core[c]
        d = np.zeros((TCH * P, K), np.float32)
        oh = np.zeros((TCH * P, P), np.float32)
        ji = np.zeros((TCH * P,), np.int64)
        for w in range(NW):
            base = w * w_ch * P
            for half, khalf, boff in ((0, k_lo, 0), (1, k_hi, k_lo * P)):
                sel = np.nonzero((win == w) & (is_hi == bool(half)))[0]
                n = len(sel)
                bs = base + boff
                d[bs:bs + n] = descr_full[s + sel]
                ji[bs:bs + n] = idx_j[s + sel] - (SPLIT if half else 0)
                oh[bs + np.arange(n), loc[sel] - w * P] = 1.0
        idx16 = np.zeros((16, TCH * 8), np.int16)
        for w in range(NW):
            for khalf, cb in ((k_lo, w * w_ch), (k_hi, w * w_ch + k_lo)):
                ni = khalf * P
                blk = ji[cb * P:cb * P + ni].reshape(ni // 16, 16).T
                idx16[:, cb * 8:cb * 8 + ni // 16] = blk.astype(np.int16)
        x0 = np.ascontiguousarray(feats[c * NA:(c + 1) * NA].T)
        in_maps.append(dict(
            x0=x0,
            descr=np.ascontiguousarray(d.T).astype(ml_dtypes.bfloat16),
            onehot=oh.astype(ml_dtypes.float8_e4m3),
            idx16=np.tile(idx16, (8, 1))))

    wall = np.zeros((B, 13, P, P), np.float32)
    wg_all = np.zeros((B, K, P), np.float32)
    biasT = np.zeros((B, 16, P), np.float32)
    brow = np.zeros((B, 2, P), np.float32)
    for b in range(B):
        wall[b, 0] = W["Wi"][b]
        wall[b, 1] = W["Wj"][b]
        for r in range(NRI):
            wall[b, 2 + r] = W["Wr1"][b, r]
            wall[b, 5 + r] = W["Wr2"][b, r]
        wall[b, 8] = W["Wout"][b]
        for r in range(NRF):
            wall[b, 9 + r] = W["Wf1"][b, r]
            wall[b, 11 + r] = W["Wf2"][b, r]
        wg_all[b] = W["Wg"][b]
        biasT[b, 0] = W["bi"][b]
        for r in range(NRI):
            biasT[b, 1 + r] = W["br1"][b, r]
            biasT[b, 4 + r] = W["br2"][b, r]
        biasT[b, 7] = W["bout"][b]
        biasT[b, 8] = W["u"][b]
        for r in range(NRF):
            biasT[b, 9 + r] = W["bf1"][b, r]
            biasT[b, 11 + r] = W["bf2"][b, r]
        biasT[b, 13] = 0.5
        biasT[b, 14] = W["bj"][b]
        brow[b, 0] = 1.0
        brow[b, 1] = W["bj"][b]
    shared = dict(wall=wall.reshape(B * 13 * P, P),
                  wg_all=wg_all.reshape(B * K, P).astype(ml_dtypes.bfloat16),
                  biasT=np.ascontiguousarray(biasT.reshape(B * 16, P).T),
                  brow=brow.reshape(1, B * 2 * P))
    for m in in_maps:
        m.update(shared)

    nc = build(k_lo, k_hi)
    res = bass_utils.run_bass_kernel_spmd(nc, in_maps, core_ids=list(range(NC)))
    global LAST_EXEC_NS, LAST_TRACE, LAST_PROFILE_JSON
    if getattr(res, "exec_time_ns", None):
        LAST_EXEC_NS = res.exec_time_ns
    if getattr(res, "instructions_and_trace", None):
        LAST_TRACE = res.instructions_and_trace[1]
    if getattr(res, "profile_json", None):
        LAST_PROFILE_JSON = res.profile_json
    out = np.empty((B, N_ATOMS, F), np.float32)
    for c in range(NC):
        slab = res.results[c]["xout"]
        out[:, c * NA:(c + 1) * NA, :] = np.transpose(slab, (0, 2, 1))
    return out
